# revision 1
# baseline (speedup 1.0000x reference)
"""Causal self-attention (RMSNorm-QK + RoPE) Trainium2 Bass kernel.

Problem: B=2, T=2048, C=1024, H=16 heads, D=64.
Sharding: 8 cores = 2 (batch) x 4 (head groups of 4 heads).
Each core computes q/k/v projections for its 4 heads, attention, and a
partial output projection (column-parallel over heads); the host sums the
4 partials per batch and transposes.

All matmuls run in float32r (TF32-like, ~13-bit mantissa, 4x fp32 matmul
speed). f32r matmul operands must be produced by rounding ops or f32r DMA;
host pre-rounds the DRAM inputs.

Per-core layouts ("T-layout" = channels on partitions, tokens free):
  projection chunks [128, 512]: row 32h+i = head h, rope-half dim i
  qT_r/kT_r  2 x [128, 2048] f32r : chunk c rows 64*(h%2)+d = head 2c+h%2
  v_r        16 x [128, 260] f32r : head h at cols 65h..65h+63, ones col
  scoresT    [s-chunk 128, t-block 512]; softmax denom = ones-column row
  yT_sb      2 x [128, 2048] f32r : pair chunk c = heads (2c, 2c+1)
Output: outT [1024, 2048] = (partial out).T per core; host sums + transposes.
"""

import sys

for _p in ("/opt/trn_rl_repo",):
    if _p not in sys.path:
        sys.path.append(_p)

import numpy as np

B, T, C = 2, 2048, 1024
H_TOT, D = 16, 64
HPC = 4               # heads per core
N_CORES = 8
P = 128               # partitions
NB = 4                # t-blocks of 512
TB = 512              # t-block size
KCH = 8               # C / 128 contraction chunks
VW = 65 * HPC         # v width with ones columns = 260
RMS_EPS = 1.1920928955078125e-07
ROPE_BASE = 10000.0

_CACHE = {}


def _build_consts():
    """Host-side constant tensors shared by all cores."""
    inv_freq = (1.0 / (ROPE_BASE ** (np.arange(0, D, 2, dtype=np.float32) / np.float32(D)))).astype(np.float32)
    pos = np.arange(T, dtype=np.float32)
    freqs = np.outer(pos, inv_freq).astype(np.float32)      # [T, 32]
    cos = np.cos(freqs).astype(np.float32)                  # [T, 32]
    sin = np.sin(freqs).astype(np.float32)
    cosr = np.ascontiguousarray(np.tile(cos.T, (HPC, 1)))   # [128, T]
    sinr = np.ascontiguousarray(np.tile(sin.T, (HPC, 1)))
    # ind32 [128, 4]: per-32-row-group summing matrix (lhsT for RMS sums)
    ind32 = np.zeros((P, HPC), dtype=np.float32)
    for p_ in range(P):
        ind32[p_, p_ // 32] = 1.0
    # bc32 [4, 128]: broadcast inv (4 heads) to 32-row groups (lhsT)
    bc32 = np.zeros((HPC, P), dtype=np.float32)
    for p_ in range(P):
        bc32[p_ // 32, p_] = 1.0
    # selpair [128, 256]: chunk c (=0,1): col m -> den row 32*(2c + m//64)
    selpair = np.zeros((P, 2 * P), dtype=np.float32)
    for c in range(2):
        for m in range(P):
            selpair[32 * (2 * c + m // 64), 128 * c + m] = 1.0
    return dict(cosr=cosr, sinr=sinr, ind32=ind32, bc32=bc32,
                selpair=selpair)


def _build_module():
    import concourse.bacc as bacc
    import concourse.mybir as mybir
    import concourse.tile as tile

    f32 = mybir.dt.float32
    f32r = mybir.dt.float32r
    Exp = mybir.ActivationFunctionType.Exp
    Ln = mybir.ActivationFunctionType.Ln
    Alu = mybir.AluOpType

    nc = bacc.Bacc("TRN2", target_bir_lowering=False, debug=False,
                   num_devices=N_CORES)

    xt_d = nc.dram_tensor("xt", [C, T], f32r, kind="ExternalInput").ap()
    wq_d = nc.dram_tensor("wq", [C, 256], f32r, kind="ExternalInput").ap()
    wk_d = nc.dram_tensor("wk", [C, 256], f32r, kind="ExternalInput").ap()
    wv_d = nc.dram_tensor("wv", [C, VW], f32r, kind="ExternalInput").ap()
    wp_d = nc.dram_tensor("wp", [256, C], f32r, kind="ExternalInput").ap()
    cosr_d = nc.dram_tensor("cosr", [P, T], f32, kind="ExternalInput").ap()
    sinr_d = nc.dram_tensor("sinr", [P, T], f32, kind="ExternalInput").ap()
    ind32_d = nc.dram_tensor("ind32", [P, HPC], f32r, kind="ExternalInput").ap()
    bc32_d = nc.dram_tensor("bc32", [HPC, P], f32r, kind="ExternalInput").ap()
    selpair_d = nc.dram_tensor("selpair", [P, 2 * P], f32r, kind="ExternalInput").ap()
    zeros_d = nc.dram_tensor("zeros", [64, T], f32r, kind="ExternalInput").ap()
    out_d = nc.dram_tensor("outT", [C, T], f32, kind="ExternalOutput").ap()

    with tile.TileContext(nc) as tc:
        with (
            tc.tile_pool(name="sb", bufs=1) as sb,
            tc.tile_pool(name="trans", bufs=2) as tr,
            tc.tile_pool(name="ps", bufs=2, space="PSUM") as ps,
        ):
            # ---- constants / weights in (direct f32r DMA) ----
            def direct_load(name, dram_slice, shape, dt=f32r):
                t_r = sb.tile(shape, dt, tag=name, name=name)
                nc.sync.dma_start(out=t_r[:], in_=dram_slice)
                return t_r

            ind32_r = direct_load("ind32r", ind32_d[:, :], [P, HPC])
            bc32_r = direct_load("bc32r", bc32_d[:, :], [HPC, P])
            selpair_r = direct_load("selpairr", selpair_d[:, :], [P, 2 * P])
            cosr_t = direct_load("cosr", cosr_d[:, :], [P, T], f32)
            sinr_t = direct_load("sinr", sinr_d[:, :], [P, T], f32)
            wq_r = [direct_load(f"wqr{k}", wq_d[k * P:(k + 1) * P, :], [P, 256])
                    for k in range(KCH)]
            wk_r = [direct_load(f"wkr{k}", wk_d[k * P:(k + 1) * P, :], [P, 256])
                    for k in range(KCH)]
            wv_r = [direct_load(f"wvr{k}", wv_d[k * P:(k + 1) * P, :], [P, VW])
                    for k in range(KCH)]
            wp_r = [direct_load(f"wpr{c}", wp_d[c * P:(c + 1) * P, :], [P, C])
                    for c in range(2)]

            # ---- persistent intermediates ----
            qT_r = [sb.tile([P, T], f32r, tag=f"qT{c}", name=f"qT{c}")
                    for c in range(2)]
            # kTe[c]: rows 0-63 = head 2c, rows 64-127 zero;
            # kTo[c]: rows 0-63 zero, rows 64-127 = head 2c+1.
            kTe = [sb.tile([P, T], f32r, tag=f"kTe{c}", name=f"kTe{c}")
                   for c in range(2)]
            kTo = [sb.tile([P, T], f32r, tag=f"kTo{c}", name=f"kTo{c}")
                   for c in range(2)]
            for c in range(2):
                nc.sync.dma_start(out=kTe[c][64:128, :], in_=zeros_d[:, :])
                nc.sync.dma_start(out=kTo[c][0:64, :], in_=zeros_d[:, :])
            v_r = [sb.tile([P, VW], f32r, tag=f"v{s}", name=f"v{s}")
                   for s in range(T // P)]
            yT_sb = [sb.tile([P, T], f32r, tag=f"yT{c}", name=f"yT{c}")
                     for c in range(2)]
            den_stack = sb.tile([P, T], f32r, tag="denstack", name="den_stack")
            nc.gpsimd.memset(den_stack[:].bitcast(f32), 1.0)
            eps_t = sb.tile([HPC, 1], f32, tag="epst", name="eps_t")
            nc.gpsimd.memset(eps_t[:], RMS_EPS)

            # ====== Phase 1+2: projections, RMS-norm, RoPE, repack ======
            for n in range(NB):
                nsl = slice(n * TB, (n + 1) * TB)
                xr_t = []
                for k in range(KCH):
                    xr = tr.tile([P, TB], f32r, tag="xr", name=f"xr{n}_{k}", bufs=9)
                    nc.sync.dma_start(out=xr[:], in_=xt_d[k * P:(k + 1) * P, nsl])
                    xr_t.append(xr)
                pq0 = ps.tile([P, TB], f32, tag="psA", name=f"pq0_{n}")
                pq1 = ps.tile([P, TB], f32, tag="psA", name=f"pq1_{n}")
                pk0 = ps.tile([P, TB], f32, tag="psB", name=f"pk0_{n}")
                pk1 = ps.tile([P, TB], f32, tag="psB", name=f"pk1_{n}")
                for k in range(KCH):
                    xr = xr_t[k]
                    st = (k == 0)
                    sp = (k == KCH - 1)
                    nc.tensor.matmul(pq0[:], lhsT=wq_r[k][:, 0:128], rhs=xr[:],
                                     start=st, stop=sp)
                    nc.tensor.matmul(pq1[:], lhsT=wq_r[k][:, 128:256], rhs=xr[:],
                                     start=st, stop=sp)
                    nc.tensor.matmul(pk0[:], lhsT=wk_r[k][:, 0:128], rhs=xr[:],
                                     start=st, stop=sp)
                    nc.tensor.matmul(pk1[:], lhsT=wk_r[k][:, 128:256], rhs=xr[:],
                                     start=st, stop=sp)
                # q/k chunks out of PSUM
                x1q = tr.tile([P, TB], f32, tag="x1q", name=f"x1q{n}", bufs=1)
                x2q = tr.tile([P, TB], f32, tag="x2q", name=f"x2q{n}", bufs=1)
                x1k = tr.tile([P, TB], f32, tag="x1k", name=f"x1k{n}", bufs=1)
                x2k = tr.tile([P, TB], f32, tag="x2k", name=f"x2k{n}", bufs=1)
                nc.vector.tensor_copy(x1q[:], pq0[:])
                nc.vector.tensor_copy(x2q[:], pq1[:])
                nc.vector.tensor_copy(x1k[:], pk0[:])
                nc.vector.tensor_copy(x2k[:], pk1[:])
                # v projections (second sub-pass over the same xr tiles)
                pv = [ps.tile([P, VW], f32, tag=("psA" if s < 2 else "psB"),
                              name=f"pv{n}_{s}") for s in range(4)]
                for k in range(KCH):
                    st = (k == 0)
                    sp = (k == KCH - 1)
                    for s_rel in range(4):
                        nc.tensor.matmul(
                            pv[s_rel][:],
                            lhsT=xr_t[k][:, s_rel * P:(s_rel + 1) * P],
                            rhs=wv_r[k][:], start=st, stop=sp)
                for s_rel in range(4):
                    vt = v_r[4 * n + s_rel]
                    nc.vector.tensor_copy(vt[:], pv[s_rel][:])
                    nc.vector.tensor_scalar(vt[:, 64:VW:65], pv[s_rel][:, 64:VW:65],
                                            0.0, 1.0, Alu.mult, Alu.add)
                # RMS-norm + RoPE + repack, per tensor
                for (x1, x2, dstT, eng) in ((x1q, x2q, qT_r, "q"),
                                            (x1k, x2k, None, "k")):
                    e = nc.vector if eng == "q" else nc.gpsimd
                    sq1 = tr.tile([P, TB], f32r, tag="tmpA", name=f"sq1{eng}{n}", bufs=1)
                    sq2 = tr.tile([P, TB], f32r, tag="tmpB", name=f"sq2{eng}{n}", bufs=1)
                    nc.gpsimd.tensor_mul(sq1[:], x1[:], x1[:])
                    nc.gpsimd.tensor_mul(sq2[:], x2[:], x2[:])
                    ps_s = ps.tile([HPC, TB], f32, tag="psA", name=f"pss{eng}{n}")
                    nc.tensor.matmul(ps_s[:], lhsT=ind32_r[:], rhs=sq1[:],
                                     start=True, stop=False)
                    nc.tensor.matmul(ps_s[:], lhsT=ind32_r[:], rhs=sq2[:],
                                     start=False, stop=True)
                    invc = tr.tile([HPC, TB], f32r, tag="invc", name=f"invc{eng}{n}")
                    nc.scalar.activation(invc[:], ps_s[:], Ln,
                                         bias=eps_t[:], scale=1.0 / 64.0)
                    nc.scalar.activation(invc[:], invc[:], Exp, scale=-0.5)
                    ps_b = ps.tile([P, TB], f32, tag="psB", name=f"psb{eng}{n}")
                    nc.tensor.matmul(ps_b[:], lhsT=bc32_r[:], rhs=invc[:],
                                     start=True, stop=True)
                    nc.vector.tensor_mul(x1[:], x1[:], ps_b[:])
                    nc.vector.tensor_mul(x2[:], x2[:], ps_b[:])
                    # rope
                    m_a = tr.tile([P, TB], f32, tag="tmpA", name=f"ma{eng}{n}", bufs=1)
                    m_b = tr.tile([P, TB], f32, tag="tmpB", name=f"mb{eng}{n}", bufs=1)
                    rc1 = tr.tile([P, TB], f32r, tag="roch1", name=f"rc1{eng}{n}", bufs=1)
                    rc2 = tr.tile([P, TB], f32r, tag="roch2", name=f"rc2{eng}{n}", bufs=1)
                    e.tensor_mul(m_a[:], x1[:], cosr_t[:, nsl])
                    e.tensor_mul(m_b[:], x2[:], sinr_t[:, nsl])
                    e.tensor_add(rc1[:], m_a[:], m_b[:])
                    m_c = tr.tile([P, TB], f32, tag="tmpA", name=f"mc{eng}{n}", bufs=1)
                    m_d = tr.tile([P, TB], f32, tag="tmpB", name=f"md{eng}{n}", bufs=1)
                    e.tensor_mul(m_c[:], x2[:], cosr_t[:, nsl])
                    e.tensor_mul(m_d[:], x1[:], sinr_t[:, nsl])
                    e.tensor_sub(rc2[:], m_c[:], m_d[:])
                    # repack: head h rows 32h..32h+32 of (rc1|rc2) ->
                    # q: qT_r[h//2] rows 64*(h%2)..; k: kTe/kTo (zero-padded)
                    for h in range(HPC):
                        if eng == "q":
                            dst = dstT[h // 2]
                            rb = 64 * (h % 2)
                        else:
                            dst = (kTe if h % 2 == 0 else kTo)[h // 2]
                            rb = 64 * (h % 2)
                        hs = slice(32 * h, 32 * h + 32)
                        nc.sync.dma_start(out=dst[rb:rb + 32, nsl], in_=rc1[hs, :])
                        nc.sync.dma_start(out=dst[rb + 32:rb + 64, nsl], in_=rc2[hs, :])

            # ================= Phase 3: attention =================
            for h in range(HPC):
                cch = h // 2
                kT_h = (kTe if h % 2 == 0 else kTo)[cch]
                rsl = slice(64 * (h % 2), 64 * (h % 2) + 64)
                pa = "psC"
                ya = "psD"
                et_tag = "expT" if h % 2 == 0 else "expT2"
                for j in range(NB):
                    jsl = slice(j * TB, (j + 1) * TB)
                    n_k = 4 * j + 4
                    Yh = ps.tile([65, TB], f32, tag=ya, name=f"Y{h}_{j}")
                    for k in range(n_k):
                        ksl = slice(k * P, (k + 1) * P)
                        st, sp = (k == 0), (k == n_k - 1)
                        r = k - 4 * j          # >=0 on diagonal blocks
                        # cols t < 128r of this block are fully masked; trim
                        # matmuls to N>=256 (f32r full-rate) and exp always.
                        mtrim = 128 * r if 0 < r <= 2 else 0
                        etrim = 128 * r if r > 0 else 0
                        msl = slice(mtrim, TB)
                        esl = slice(etrim, TB)
                        S0 = ps.tile([P, TB], f32, tag=pa, name=f"S{h}_{j}_{k}")
                        nc.tensor.matmul(S0[:, msl], lhsT=kT_h[:, ksl],
                                         rhs=qT_r[cch][:, j * TB + mtrim:(j + 1) * TB],
                                         start=True, stop=True)
                        e0 = tr.tile([P, TB], f32r, tag=et_tag,
                                     name=f"e{h}_{j}_{k}", bufs=3)
                        nc.scalar.activation(e0[:, esl], S0[:, esl], Exp, scale=0.125)
                        if r >= 0:  # diagonal: apply causal mask (zero-fills left)
                            e0m = tr.tile([P, TB], f32r, tag=et_tag,
                                          name=f"em{h}_{j}_{k}", bufs=3)
                            nc.gpsimd.affine_select(
                                out=e0m[:], in_=e0[:], pattern=[[1, TB]],
                                compare_op=Alu.is_ge, fill=0.0,
                                base=-128 * r, channel_multiplier=-1)
                            e0 = e0m
                        nc.tensor.matmul(Yh[:, msl], lhsT=v_r[k][:, 65 * h:65 * h + 65],
                                         rhs=e0[:, msl], start=st, stop=sp)
                    # copy out: y rows + den row (SBUF bounce; DMA shifts rows)
                    yb = tr.tile([65, TB], f32r, tag="cpbuf", name=f"yb{h}_{j}",
                                 bufs=3, padded_shape=[P, TB])
                    nc.vector.tensor_copy(yb[:], Yh[:])
                    nc.sync.dma_start(out=yT_sb[cch][rsl, jsl], in_=yb[0:64, :])
                    nc.sync.dma_start(out=den_stack[32 * h:32 * h + 1, jsl],
                                      in_=yb[64:65, :])

            # ================= Phase 4: normalize + out-projection ======
            # invden = exp(-ln(den)) on rows 0,32,64,96 (others memset to 1)
            invden_r = sb.tile([P, T], f32r, tag="invden", name="invden_r")
            nc.scalar.activation(den_stack[:], den_stack[:], Ln)
            nc.scalar.activation(invden_r[:], den_stack[:], Exp, scale=-1.0)
            for c in range(2):
                for n in range(NB):
                    nsl = slice(n * TB, (n + 1) * TB)
                    ps_i = ps.tile([P, TB], f32, tag="psA", name=f"psi{c}{n}")
                    nc.tensor.matmul(ps_i[:], lhsT=selpair_r[:, c * P:(c + 1) * P],
                                     rhs=invden_r[:, nsl], start=True, stop=True)
                    nc.vector.tensor_mul(yT_sb[c][:, nsl], yT_sb[c][:, nsl], ps_i[:])
            for o in range(8):
                osl = slice(o * P, (o + 1) * P)
                for n in range(NB):
                    nsl = slice(n * TB, (n + 1) * TB)
                    po = ps.tile([P, TB], f32, tag="psB", name=f"po{o}_{n}")
                    nc.tensor.matmul(po[:], lhsT=wp_r[0][:, osl], rhs=yT_sb[0][:, nsl],
                                     start=True, stop=False)
                    nc.tensor.matmul(po[:], lhsT=wp_r[1][:, osl], rhs=yT_sb[1][:, nsl],
                                     start=False, stop=True)
                    ob = tr.tile([P, TB], f32, tag="cpbuf", name=f"ob{o}_{n}", bufs=3)
                    nc.vector.tensor_copy(ob[:], po[:])
                    nc.sync.dma_start(out=out_d[osl, nsl], in_=ob[:])

    nc.compile()
    return nc


def _get_module():
    if "nc" not in _CACHE:
        _CACHE["nc"] = _build_module()
        _CACHE["consts"] = _build_consts()
    return _CACHE["nc"], _CACHE["consts"]


def _round_f32r(a, bits=10):
    u = np.ascontiguousarray(a, dtype=np.float32).view(np.uint32).astype(np.uint64)
    u = (u + (1 << (bits - 1))) & ~np.uint64((1 << bits) - 1)
    return np.minimum(u, 0xFFFFFFFF).astype(np.uint32).view(np.float32)


def _core_inputs(x, w_q, w_k, w_v, w_proj, core):
    """Build the per-core input map (numpy, host-side sharding)."""
    b = core // 4
    g = core % 4
    heads = [4 * g + j for j in range(HPC)]

    xt = _round_f32r(np.ascontiguousarray(x[b].T))        # [C, T]

    perm = np.empty(256, dtype=np.int64)
    for m in range(128):
        perm[m] = 64 * heads[m // 32] + (m % 32)             # x1 half
        perm[128 + m] = 64 * heads[m // 32] + 32 + (m % 32)  # x2 half
    wq = _round_f32r(np.ascontiguousarray(w_q[perm, :].T))   # [C, 256]
    wk = _round_f32r(np.ascontiguousarray(w_k[perm, :].T))

    # v weights with zero columns at 65h+64 (device writes the ones there)
    wv_aug = np.zeros((C, VW), dtype=np.float32)
    for j in range(HPC):
        wv_aug[:, 65 * j:65 * j + 64] = w_v[64 * heads[j]:64 * heads[j] + 64, :].T
    wv = _round_f32r(wv_aug)

    vperm = np.empty(256, dtype=np.int64)
    for m in range(256):
        vperm[m] = 64 * heads[m // 64] + (m % 64)
    wp = _round_f32r(np.ascontiguousarray(w_proj[:, vperm].T))  # [256, C]

    zeros = np.zeros((64, T), dtype=np.float32)
    return dict(xt=xt, wq=wq, wk=wk, wv=wv, wp=wp, zeros=zeros)


def kernel(x, w_q, w_k, w_v, w_proj, _trace=False, _trace_cores=None):
    from concourse.bass_utils import run_bass_kernel_spmd

    nc, consts = _get_module()
    x = np.asarray(x, dtype=np.float32)
    in_maps = []
    for core in range(N_CORES):
        m = _core_inputs(np.asarray(x), np.asarray(w_q), np.asarray(w_k),
                         np.asarray(w_v), np.asarray(w_proj), core)
        m.update(consts)
        in_maps.append(m)

    res = run_bass_kernel_spmd(nc, in_maps, list(range(N_CORES)),
                               trace=_trace, trace_cores=_trace_cores)
    outs = [res.results[c]["outT"] for c in range(N_CORES)]
    out = np.empty((B, T, C), dtype=np.float32)
    for b in range(B):
        acc = outs[4 * b].astype(np.float32)
        for g in range(1, 4):
            acc = acc + outs[4 * b + g]
        out[b] = acc.T
    if _trace:
        kernel._last_exec_time_ns = res.exec_time_ns
        kernel._last_results = res
    return out



# revision 5
# speedup vs baseline: 1.3479x; 1.3479x over previous
"""Causal self-attention (RMSNorm-QK + RoPE) Trainium2 Bass kernel.

Problem: B=2, T=2048, C=1024, H=16 heads, D=64.
Sharding: 8 cores = 2 (batch) x 4 (head groups of 4 heads).
Each core computes q/k/v projections for its 4 heads, attention, and a
partial output projection (column-parallel over heads); the host sums the
4 partials per batch and transposes.

All matmuls / elementwise run in float16 (tolerance is 2e-2; fp16 keeps the
softmax-exponent error ~1e-3).  PSUM accumulation is fp32.

Per-core layouts ("attention layout", channels on partitions, tokens free):
  q/k/v projections per chunk c in {0,1}: partitions = [head 2c dims 0..63,
  head 2c+1 dims 0..63].  RoPE pairs (d, d+32) live at partitions p, p^32;
  the rotate-half operand is produced by 4 SBUF->SBUF partition-swap DMAs.
  K is NOT normalized; rms(k) is folded into the softmax exp as a
  per-partition (= per-key-token) activation scale, computed in transposed
  form ([token, head] tiles) directly via tiny N=2 matmuls.
  v_r  16 x [128, 260] f16 : head h at cols 65h..65h+63, ones col at 65h+64
  scores/exp per (head, j-block 512, s-chunk 128), causally trimmed to
  128-col granularity; diagonal 128x128 masked in-place with affine_select.
  Y psum [65, 512]: rows 0..63 = sum(exp * v), row 64 = softmax denominator.
Output: outT [1024, 2048] f16 = (partial out).T per core; host sums 4
partials per batch (fp32) and transposes.
"""

import sys

for _p in ("/opt/trn_rl_repo",):
    if _p not in sys.path:
        sys.path.append(_p)

import numpy as np

B, T, C = 2, 2048, 1024
H_TOT, D = 16, 64
HPC = 4               # heads per core
N_CORES = 8
P = 128               # partitions
NB = 4                # t-blocks of 512
TB = 512              # t-block size
KCH = 8               # C / 128 contraction chunks
VW = 65 * HPC         # v width with ones columns = 260
RMS_EPS = 1.1920928955078125e-07
ROPE_BASE = 10000.0

_CACHE = {}


def _patch_act_tables():
    """Restrict ln/exp to the combined act-table set so bass's greedy
    first-match table pass emits a single LoadActFuncSet instead of
    ping-ponging (1.28us per reload).  Set indices are unchanged, only the
    (cached) set contents seen by the placement pass."""
    import concourse.bacc as bacc
    import concourse.hw_specs as hw_specs
    import concourse.mybir as mybir

    if getattr(bacc, "_act_tables_patched", False):
        return
    orig = hw_specs.get_activation_tables

    def patched(arch):
        tabs = dict(orig(arch))
        out = {}
        for name, s in tabs.items():
            s = set(s)
            if name != "natural_log_exp_and_others":
                s.discard(mybir.ActivationFunctionType.Ln)
                s.discard(mybir.ActivationFunctionType.Exp)
            out[name] = s
        return out

    bacc.get_activation_tables = patched
    bacc._act_tables_patched = True


def _build_consts():
    """Host-side constant tensors shared by all cores (fp16)."""
    inv_freq = (1.0 / (ROPE_BASE ** (np.arange(0, D, 2, dtype=np.float32) / np.float32(D)))).astype(np.float32)
    pos = np.arange(T, dtype=np.float32)
    freqs = np.outer(pos, inv_freq).astype(np.float32)      # [T, 32]
    cos = np.cos(freqs).astype(np.float32).T                # [32, T]
    sin = np.sin(freqs).astype(np.float32).T
    # cos2[p] = cos[p%32]; sin2[p] = +sin[p%32] for (p//32)%2==0 else -sin
    cos2 = np.tile(cos, (4, 1)).astype(np.float16)          # [128, T]
    sin2 = np.empty((P, T), dtype=np.float32)
    for g in range(4):
        sgn = 1.0 if g % 2 == 0 else -1.0
        sin2[32 * g:32 * g + 32] = sgn * sin
    sin2 = sin2.astype(np.float16)
    ind2 = np.zeros((P, 2), dtype=np.float16)               # col j: rows 64j..64j+63
    ind2[0:64, 0] = 1.0
    ind2[64:128, 1] = 1.0
    bc64 = np.zeros((2, P), dtype=np.float16)               # row j: cols 64j..
    bc64[0, 0:64] = 1.0
    bc64[1, 64:128] = 1.0
    return dict(cos2=cos2, sin2=sin2, ind2=ind2, bc64=bc64)


def _build_module():
    _patch_act_tables()
    import concourse.bacc as bacc
    import concourse.mybir as mybir
    import concourse.tile as tile

    f32 = mybir.dt.float32
    f32r = mybir.dt.float32r
    f16 = mybir.dt.float16
    Exp = mybir.ActivationFunctionType.Exp
    Ln = mybir.ActivationFunctionType.Ln
    Copy = mybir.ActivationFunctionType.Copy
    Alu = mybir.AluOpType

    nc = bacc.Bacc("TRN2", target_bir_lowering=False, debug=False,
                   num_devices=N_CORES)

    xt_d = nc.dram_tensor("xt", [C, T], f16, kind="ExternalInput").ap()
    wq_d = nc.dram_tensor("wq", [C, 256], f16, kind="ExternalInput").ap()
    wk_d = nc.dram_tensor("wk", [C, 256], f16, kind="ExternalInput").ap()
    wv_d = nc.dram_tensor("wv", [C, VW], f16, kind="ExternalInput").ap()
    wp_d = nc.dram_tensor("wp", [256, C], f16, kind="ExternalInput").ap()
    cos2_d = nc.dram_tensor("cos2", [P, T], f16, kind="ExternalInput").ap()
    sin2_d = nc.dram_tensor("sin2", [P, T], f16, kind="ExternalInput").ap()
    ind2_d = nc.dram_tensor("ind2", [P, 2], f16, kind="ExternalInput").ap()
    bc64_d = nc.dram_tensor("bc64", [2, P], f16, kind="ExternalInput").ap()
    out_d = nc.dram_tensor("outT", [C, T], f16, kind="ExternalOutput").ap()

    with tile.TileContext(nc) as tc:
        with (
            tc.tile_pool(name="sb", bufs=1) as sb,
            tc.tile_pool(name="tr", bufs=2) as tr,
            tc.tile_pool(name="ps", bufs=2, space="PSUM") as ps,
        ):
            # ---------------- persistent tiles + loads ----------------
            def load(name, dram_slice, shape, dt=f16):
                t = sb.tile(shape, dt, tag=name, name=name)
                nc.sync.dma_start(out=t[:], in_=dram_slice)
                return t

            wq_t = [load(f"wq{k}", wq_d[k * P:(k + 1) * P, :], [P, 256])
                    for k in range(KCH)]
            wk_t = [load(f"wk{k}", wk_d[k * P:(k + 1) * P, :], [P, 256])
                    for k in range(KCH)]
            # x halves: x_t[k][half] covers cols half*1024 .. +1024 (2 blocks)
            x_t = [[sb.tile([P, 2 * TB], f16, tag=f"x{k}_{hf}", name=f"x{k}_{hf}")
                    for hf in range(2)] for k in range(KCH)]
            for k in range(KCH):
                nc.sync.dma_start(out=x_t[k][0][:],
                                  in_=xt_d[k * P:(k + 1) * P, 0:2 * TB])
            cos2_t = load("cos2", cos2_d[:, :], [P, T])
            sin2_t = load("sin2", sin2_d[:, :], [P, T])
            ind2_t = load("ind2", ind2_d[:, :], [P, 2])
            bc64_t = load("bc64", bc64_d[:, :], [2, P])
            wv_t = [load(f"wv{k}", wv_d[k * P:(k + 1) * P, :], [P, VW])
                    for k in range(KCH)]
            wp_t = [load(f"wp{c}", wp_d[c * P:(c + 1) * P, :], [P, C])
                    for c in range(2)]

            ones65 = sb.tile([65, 64], f32r, tag="ones65", name="ones65")
            nc.gpsimd.memset(ones65[:].bitcast(f32), 1.0)
            epsq = sb.tile([2, 1], f32, tag="epsq", name="epsq")
            nc.gpsimd.memset(epsq[:], RMS_EPS)
            epsk = sb.tile([P, 1], f32, tag="epsk", name="epsk")
            nc.gpsimd.memset(epsk[:], 64.0 * RMS_EPS)

            # persistent intermediates
            rcq_t = [sb.tile([P, T], f16, tag=f"rcq{c}", name=f"rcq{c}")
                     for c in range(2)]
            rck_t = [sb.tile([P, T], f16, tag=f"rck{c}", name=f"rck{c}")
                     for c in range(2)]
            yT_t = [sb.tile([P, T], f16, tag=f"yT{c}", name=f"yT{c}")
                    for c in range(2)]
            v_t = [sb.tile([P, VW], f16, tag=f"v{s}", name=f"v{s}")
                   for s in range(T // P)]
            invkT_t = [[sb.tile([P, 8], f32, tag=f"ikT{n}_{c}", name=f"ikT{n}_{c}")
                        for c in range(2)] for n in range(NB)]

            # ---------------- phase 1: proj + rms + rope ----------------
            def p1(n):
                half, tloc = n // 2, (n % 2) * TB
                nsl = slice(n * TB, (n + 1) * TB)
                xr = lambda k: x_t[k][half][:, tloc:tloc + TB]

                # ---- q ----
                pq = [ps.tile([P, TB], f32, tag="p1", name=f"pq{n}_{c}", bufs=2)
                      for c in range(2)]
                for c in range(2):
                    for k in range(KCH):
                        nc.tensor.matmul(pq[c][:], lhsT=wq_t[k][:, c * P:(c + 1) * P],
                                         rhs=xr(k), start=(k == 0), stop=(k == KCH - 1))
                xq = tr.tile([P, 2 * TB], f16, tag="xm", name=f"xq{n}", bufs=2)
                for c in range(2):
                    nc.scalar.activation(xq[:, c * TB:(c + 1) * TB], pq[c][:], Copy)
                sq = tr.tile([P, 2 * TB], f16, tag="sqm", name=f"sq{n}", bufs=2)
                nc.vector.tensor_mul(sq[:], xq[:], xq[:])
                xn = tr.tile([P, 2 * TB], f16, tag="xnm", name=f"xn{n}", bufs=2)
                for c in range(2):
                    csl = slice(c * TB, (c + 1) * TB)
                    msum = ps.tile([2, TB], f32, tag="pst", name=f"ms{n}_{c}", bufs=1)
                    nc.tensor.matmul(msum[:], lhsT=ind2_t[:], rhs=sq[:, csl],
                                     start=True, stop=True)
                    invr = tr.tile([2, TB], f16, tag="invr", name=f"ivr{n}_{c}",
                                   bufs=2)
                    nc.scalar.activation(invr[:], msum[:], Ln,
                                         bias=epsq[:], scale=1.0 / 64.0)
                    nc.scalar.activation(invr[:], invr[:], Exp, scale=-0.5)
                    inv128 = ps.tile([P, TB], f32, tag="p1", name=f"iv{n}_{c}", bufs=2)
                    nc.tensor.matmul(inv128[:], lhsT=bc64_t[:], rhs=invr[:],
                                     start=True, stop=True)
                    nc.vector.tensor_mul(xn[:, csl], xq[:, csl], inv128[:])
                xnsw = tr.tile([P, 2 * TB], f16, tag="xsw", name=f"xnsw{n}", bufs=2)
                for g in range(2):
                    a, b = 64 * g, 64 * g + 32
                    nc.sync.dma_start(out=xnsw[a:a + 32, :], in_=xn[b:b + 32, :])
                    nc.sync.dma_start(out=xnsw[b:b + 32, :], in_=xn[a:a + 32, :])
                for c in range(2):
                    csl = slice(c * TB, (c + 1) * TB)
                    t1 = tr.tile([P, TB], f16, tag="t12", name=f"t1q{n}_{c}", bufs=3)
                    t2 = tr.tile([P, TB], f16, tag="t12", name=f"t2q{n}_{c}", bufs=3)
                    nc.vector.tensor_mul(t1[:], xn[:, csl], cos2_t[:, nsl])
                    nc.vector.tensor_mul(t2[:], xnsw[:, csl], sin2_t[:, nsl])
                    nc.vector.tensor_add(rcq_t[c][:, nsl], t1[:], t2[:])

                # ---- k ----
                pk = [ps.tile([P, TB], f32, tag="p1", name=f"pk{n}_{c}", bufs=2)
                      for c in range(2)]
                for c in range(2):
                    for k in range(KCH):
                        nc.tensor.matmul(pk[c][:], lhsT=wk_t[k][:, c * P:(c + 1) * P],
                                         rhs=xr(k), start=(k == 0), stop=(k == KCH - 1))
                xk = tr.tile([P, 2 * TB], f16, tag="xm", name=f"xk{n}", bufs=2)
                for c in range(2):
                    nc.scalar.activation(xk[:, c * TB:(c + 1) * TB], pk[c][:], Copy)
                sqk = tr.tile([P, 2 * TB], f16, tag="sqm", name=f"sqk{n}", bufs=2)
                nc.vector.tensor_mul(sqk[:], xk[:], xk[:])
                for c in range(2):
                    kst = ps.tile([P, 8], f32, tag="pst", name=f"kst{n}_{c}", bufs=1)
                    for sc in range(4):
                        nc.tensor.matmul(kst[:, 2 * sc:2 * sc + 2],
                                         lhsT=sqk[:, c * TB + sc * P:c * TB + (sc + 1) * P],
                                         rhs=ind2_t[:], start=True, stop=True)
                    # invkT = exp(-0.5*ln(sum + 64*eps)) = rsqrt(mean+eps)/8
                    nc.scalar.activation(invkT_t[n][c][:], kst[:], Ln,
                                         bias=epsk[:])
                    nc.scalar.activation(invkT_t[n][c][:], invkT_t[n][c][:], Exp,
                                         scale=-0.5)
                xksw = tr.tile([P, 2 * TB], f16, tag="xsw", name=f"xksw{n}", bufs=2)
                for g in range(2):
                    a, b = 64 * g, 64 * g + 32
                    nc.sync.dma_start(out=xksw[a:a + 32, :], in_=xk[b:b + 32, :])
                    nc.sync.dma_start(out=xksw[b:b + 32, :], in_=xk[a:a + 32, :])
                for c in range(2):
                    csl = slice(c * TB, (c + 1) * TB)
                    t1 = tr.tile([P, TB], f16, tag="t12", name=f"t1k{n}_{c}", bufs=3)
                    t2 = tr.tile([P, TB], f16, tag="t12", name=f"t2k{n}_{c}", bufs=3)
                    nc.vector.tensor_mul(t1[:], xk[:, csl], cos2_t[:, nsl])
                    nc.vector.tensor_mul(t2[:], xksw[:, csl], sin2_t[:, nsl])
                    nc.vector.tensor_add(rck_t[c][:, nsl], t1[:], t2[:])

                # ---- v ----
                for s_rel in range(4):
                    pv = ps.tile([P, VW], f32, tag="p1", name=f"pv{n}_{s_rel}", bufs=2)
                    for k in range(KCH):
                        nc.tensor.matmul(pv[:],
                                         lhsT=xr(k)[:, s_rel * P:(s_rel + 1) * P],
                                         rhs=wv_t[k][:], start=(k == 0),
                                         stop=(k == KCH - 1))
                    vt = v_t[4 * n + s_rel]
                    nc.vector.tensor_copy(vt[:], pv[:])
                    nc.vector.tensor_scalar(vt[:, 64:VW:65], pv[:, 64:VW:65],
                                            0.0, 1.0, Alu.mult, Alu.add)

            # ---------------- attention ----------------
            def attn_head(h, j):
                cch, half = h // 2, h % 2
                rsl = slice(64 * half, 64 * half + 64)
                n_k = 4 * (j + 1)
                jsl = slice(j * TB, (j + 1) * TB)
                Y = ps.tile([65, TB], f32, tag="py", name=f"Y{h}_{j}", bufs=2)
                for k in range(n_k):
                    r = k - 4 * j
                    mt = 128 * r if r > 0 else 0
                    S = ps.tile([P, TB], f32, tag="ps", name=f"S{h}_{j}_{k}", bufs=2)
                    nc.tensor.matmul(
                        S[:, mt:], lhsT=rck_t[cch][rsl, k * P:(k + 1) * P],
                        rhs=rcq_t[cch][rsl, j * TB + mt:(j + 1) * TB],
                        start=True, stop=True)
                    e0 = tr.tile([P, TB], f16, tag="e0", name=f"e{h}_{j}_{k}",
                                 bufs=3)
                    nc.scalar.activation(
                        e0[:, mt:], S[:, mt:], Exp,
                        scale=invkT_t[k // 4][cch][:, 2 * (k % 4) + half:
                                                   2 * (k % 4) + half + 1])
                    if r >= 0:
                        nc.gpsimd.affine_select(
                            out=e0[:, 128 * r:128 * r + 128],
                            in_=e0[:, 128 * r:128 * r + 128],
                            pattern=[[1, 128]], compare_op=Alu.is_ge,
                            fill=0.0, base=0, channel_multiplier=-1)
                    nc.tensor.matmul(Y[:, mt:], lhsT=v_t[k][:, 65 * h:65 * h + 65],
                                     rhs=e0[:, mt:], start=(k == 0),
                                     stop=(k == n_k - 1))
                # normalize tail
                invden = tr.tile([65, TB], f32r, tag="inb", name=f"ivd{h}_{j}",
                                 bufs=2, padded_shape=[P, TB])
                with nc.allow_low_precision("softmax denominator reciprocal"):
                    nc.vector.reciprocal(invden[64:65, :], Y[64:65, :])
                bcD = ps.tile([64, TB], f32, tag="pb", name=f"bcD{h}_{j}", bufs=1)
                nc.tensor.matmul(bcD[:], lhsT=ones65[64:65, 0:64],
                                 rhs=invden[64:65, :], start=True, stop=True)
                yraw = tr.tile([64, TB], f16, tag="yrw", name=f"yr{h}_{j}",
                               bufs=2, padded_shape=[P, TB])
                nc.scalar.activation(yraw[:], Y[0:64, :], Copy)
                yn = tr.tile([64, TB], f16, tag="ynm", name=f"yn{h}_{j}",
                             bufs=2, padded_shape=[P, TB])
                nc.vector.tensor_mul(yn[:], yraw[:], bcD[:])
                nc.sync.dma_start(out=yT_t[cch][rsl, jsl], in_=yn[:])

            # ---------------- out-projection ----------------
            def p4(j):
                jsl = slice(j * TB, (j + 1) * TB)
                for o in range(8):
                    osl = slice(o * P, (o + 1) * P)
                    po = ps.tile([P, TB], f32, tag="p1", name=f"po{j}_{o}", bufs=2)
                    nc.tensor.matmul(po[:], lhsT=wp_t[0][:, osl],
                                     rhs=yT_t[0][:, jsl], start=True, stop=False)
                    nc.tensor.matmul(po[:], lhsT=wp_t[1][:, osl],
                                     rhs=yT_t[1][:, jsl], start=False, stop=True)
                    ob = tr.tile([P, TB], f16, tag="ob", name=f"ob{j}_{o}", bufs=3)
                    nc.vector.tensor_copy(ob[:], po[:])
                    nc.sync.dma_start(out=out_d[osl, jsl], in_=ob[:])

            # ---------------- schedule ----------------
            p1(0)
            for k in range(KCH):     # x halves for blocks 2,3
                nc.sync.dma_start(out=x_t[k][1][:],
                                  in_=xt_d[k * P:(k + 1) * P, 2 * TB:4 * TB])
            for w in range(1, NB + 1):
                j = w - 1
                for h in range(HPC):
                    attn_head(h, j)
                if w >= 2:
                    p4(w - 2)        # out-proj one window behind
                if w < NB:
                    p1(w)
            p4(NB - 1)

    nc.compile()
    return nc


def _get_module():
    if "nc" not in _CACHE:
        _CACHE["nc"] = _build_module()
        _CACHE["consts"] = _build_consts()
    return _CACHE["nc"], _CACHE["consts"]


def _core_inputs(x, w_q, w_k, w_v, w_proj, core):
    """Build the per-core input map (numpy fp16, host-side sharding)."""
    b = core // 4
    g = core % 4
    heads = [4 * g + j for j in range(HPC)]

    xt = np.ascontiguousarray(x[b].T).astype(np.float16)     # [C, T]

    # attention-layout column perm: col m of chunk c -> head 2c+(m//64), dim m%64
    perm = np.empty(256, dtype=np.int64)
    for m in range(256):
        c, mm = m // 128, m % 128
        perm[m] = 64 * heads[2 * c + mm // 64] + (mm % 64)
    wq = np.ascontiguousarray(w_q[perm, :].T).astype(np.float16)   # [C, 256]
    wk = np.ascontiguousarray(w_k[perm, :].T).astype(np.float16)

    wv_aug = np.zeros((C, VW), dtype=np.float32)
    for j in range(HPC):
        wv_aug[:, 65 * j:65 * j + 64] = w_v[64 * heads[j]:64 * heads[j] + 64, :].T
    wv = wv_aug.astype(np.float16)

    wp = np.ascontiguousarray(w_proj[:, perm].T).astype(np.float16)  # [256, C]

    return dict(xt=xt, wq=wq, wk=wk, wv=wv, wp=wp)


def kernel(x, w_q, w_k, w_v, w_proj, _trace=False, _trace_cores=None):
    from concourse.bass_utils import run_bass_kernel_spmd

    nc, consts = _get_module()
    x = np.asarray(x, dtype=np.float32)
    in_maps = []
    for core in range(N_CORES):
        m = _core_inputs(np.asarray(x), np.asarray(w_q), np.asarray(w_k),
                         np.asarray(w_v), np.asarray(w_proj), core)
        m.update(consts)
        in_maps.append(m)

    res = run_bass_kernel_spmd(nc, in_maps, list(range(N_CORES)),
                               trace=_trace, trace_cores=_trace_cores)
    outs = [res.results[c]["outT"] for c in range(N_CORES)]
    out = np.empty((B, T, C), dtype=np.float32)
    for b in range(B):
        acc = outs[4 * b].astype(np.float32)
        for g in range(1, 4):
            acc = acc + outs[4 * b + g].astype(np.float32)
        out[b] = acc.T
    if _trace:
        kernel._last_exec_time_ns = res.exec_time_ns
        kernel._last_results = res
    return out


# revision 7
# speedup vs baseline: 1.4934x; 1.1079x over previous
"""Causal self-attention (RMSNorm-QK + RoPE) Trainium2 Bass kernel.

Problem: B=2, T=2048, C=1024, H=16 heads, D=64.
Sharding: 8 cores = 2 (batch) x 4 (head groups of 4 heads).
Each core computes q/k/v projections for its 4 heads, attention, and a
partial output projection (column-parallel over heads); the host sums the
4 partials per batch and transposes.

All matmuls / elementwise run in float16 (tolerance is 2e-2; fp16 keeps the
softmax-exponent error ~1e-3).  PSUM accumulation is fp32.

Layout ("attention layout", channels on partitions, tokens free):
  q/k/v per chunk c in {0,1}: partitions = [head 2c d0..63, head 2c+1 d0..63].
  RoPE pairs (d, d+32) live at partitions p, p^32; the rotate-half operand
  comes from 4 SBUF->SBUF partition-swap DMAs.  K is NOT normalized; rms(k)
  is folded into the softmax exp as a per-partition (per-key-token)
  activation scale computed in transposed [token, head] form via N=2 matmuls.
  v 16 x [128, 260] f16: head h at cols 65h.., ones col at 65h+64 so the
  PV matmul accumulates the softmax denominator in Y row 64.

Pipeline: emission interleaves attention(j) heads with projection block
j+1 pieces (attention is Scalar-bound, projections Tensor-bound); the
S->exp->PV chain is software-pipelined (S_{k+1} issued before PV_k) so the
PE never waits on the exp; denominators of the 4 heads are batched into one
[4, T-block] Ln/Exp reciprocal on Scalar; out-projection runs one window
behind attention.
"""

import sys

for _p in ("/opt/trn_rl_repo",):
    if _p not in sys.path:
        sys.path.append(_p)

import numpy as np

B, T, C = 2, 2048, 1024
H_TOT, D = 16, 64
HPC = 4               # heads per core
N_CORES = 8
P = 128               # partitions
NB = 4                # t-blocks of 512
TB = 512              # t-block size
KCH = 8               # C / 128 contraction chunks
VW = 65 * HPC         # v width with ones columns = 260
RMS_EPS = 1.1920928955078125e-07
ROPE_BASE = 10000.0

_CACHE = {}


def _patch_act_tables():
    """Restrict ln/exp to the combined act-table set so bass's greedy
    first-match table pass emits a single LoadActFuncSet instead of
    ping-ponging (1.28us per reload).  Set indices are unchanged, only the
    (cached) set contents seen by the placement pass."""
    import concourse.bacc as bacc
    import concourse.hw_specs as hw_specs
    import concourse.mybir as mybir

    if getattr(bacc, "_act_tables_patched", False):
        return
    orig = hw_specs.get_activation_tables

    def patched(arch):
        tabs = dict(orig(arch))
        out = {}
        for name, s in tabs.items():
            s = set(s)
            if name != "natural_log_exp_and_others":
                s.discard(mybir.ActivationFunctionType.Ln)
                s.discard(mybir.ActivationFunctionType.Exp)
            out[name] = s
        return out

    bacc.get_activation_tables = patched
    bacc._act_tables_patched = True


def _build_consts():
    """Host-side constant tensors shared by all cores (fp16)."""
    inv_freq = (1.0 / (ROPE_BASE ** (np.arange(0, D, 2, dtype=np.float32) / np.float32(D)))).astype(np.float32)
    pos = np.arange(T, dtype=np.float32)
    freqs = np.outer(pos, inv_freq).astype(np.float32)      # [T, 32]
    cos = np.cos(freqs).astype(np.float32).T                # [32, T]
    sin = np.sin(freqs).astype(np.float32).T
    # cos2[p] = cos[p%32]; sin2[p] = +sin[p%32] for (p//32)%2==0 else -sin
    cos2 = np.tile(cos, (4, 1)).astype(np.float16)          # [128, T]
    sin2 = np.empty((P, T), dtype=np.float32)
    for g in range(4):
        sgn = 1.0 if g % 2 == 0 else -1.0
        sin2[32 * g:32 * g + 32] = sgn * sin
    sin2 = sin2.astype(np.float16)
    ind2 = np.zeros((P, 2), dtype=np.float16)               # col j: rows 64j..64j+63
    ind2[0:64, 0] = 1.0
    ind2[64:128, 1] = 1.0
    bc64 = np.zeros((2, P), dtype=np.float16)               # row j: cols 64j..
    bc64[0, 0:64] = 1.0
    bc64[1, 64:128] = 1.0
    sel4 = np.zeros((HPC, 256), dtype=np.float16)           # row h -> cols 64h..
    for h in range(HPC):
        sel4[h, 64 * h:64 * h + 64] = 1.0
    return dict(cos2=cos2, sin2=sin2, ind2=ind2, bc64=bc64, sel4=sel4)


def _build_module():
    _patch_act_tables()
    import concourse.bacc as bacc
    import concourse.mybir as mybir
    import concourse.tile as tile

    f32 = mybir.dt.float32
    f16 = mybir.dt.float16
    Exp = mybir.ActivationFunctionType.Exp
    Ln = mybir.ActivationFunctionType.Ln
    Copy = mybir.ActivationFunctionType.Copy
    Alu = mybir.AluOpType

    nc = bacc.Bacc("TRN2", target_bir_lowering=False, debug=False,
                   num_devices=N_CORES)

    xt_d = nc.dram_tensor("xt", [C, T], f16, kind="ExternalInput").ap()
    wq_d = nc.dram_tensor("wq", [C, 256], f16, kind="ExternalInput").ap()
    wk_d = nc.dram_tensor("wk", [C, 256], f16, kind="ExternalInput").ap()
    wv_d = nc.dram_tensor("wv", [C, VW], f16, kind="ExternalInput").ap()
    wp_d = nc.dram_tensor("wp", [256, C], f16, kind="ExternalInput").ap()
    cos2_d = nc.dram_tensor("cos2", [P, T], f16, kind="ExternalInput").ap()
    sin2_d = nc.dram_tensor("sin2", [P, T], f16, kind="ExternalInput").ap()
    ind2_d = nc.dram_tensor("ind2", [P, 2], f16, kind="ExternalInput").ap()
    bc64_d = nc.dram_tensor("bc64", [2, P], f16, kind="ExternalInput").ap()
    sel4_d = nc.dram_tensor("sel4", [HPC, 256], f16, kind="ExternalInput").ap()
    out_d = nc.dram_tensor("outT", [C, T], f16, kind="ExternalOutput").ap()

    with tile.TileContext(nc) as tc:
        with (
            tc.tile_pool(name="sb", bufs=1) as sb,
            tc.tile_pool(name="tr", bufs=2) as tr,
            tc.tile_pool(name="ps", bufs=2, space="PSUM") as ps,
        ):
            # ---------------- persistent tiles + loads ----------------
            def load(name, dram_slice, shape, dt=f16):
                t = sb.tile(shape, dt, tag=name, name=name)
                nc.sync.dma_start(out=t[:], in_=dram_slice)
                return t

            ind2_t = load("ind2", ind2_d[:, :], [P, 2])
            bc64_t = load("bc64", bc64_d[:, :], [2, P])
            sel4_t = load("sel4", sel4_d[:, :], [HPC, 256])
            wq_t = [load(f"wq{k}", wq_d[k * P:(k + 1) * P, :], [P, 256])
                    for k in range(KCH)]
            wk_t = [load(f"wk{k}", wk_d[k * P:(k + 1) * P, :], [P, 256])
                    for k in range(KCH)]
            # x per-block tiles, loaded from the (otherwise idle) Scalar DGE
            x_t = [[sb.tile([P, TB], f16, tag=f"x{k}_{n}", name=f"x{k}_{n}")
                    for n in range(NB)] for k in range(KCH)]

            def load_x(n):
                for k in range(KCH):
                    nc.scalar.dma_start(out=x_t[k][n][:],
                                        in_=xt_d[k * P:(k + 1) * P,
                                                 n * TB:(n + 1) * TB])

            load_x(0)
            cos2_t = load("cos2", cos2_d[:, :], [P, T])
            sin2_t = load("sin2", sin2_d[:, :], [P, T])
            load_x(1)
            wv_t = [load(f"wv{k}", wv_d[k * P:(k + 1) * P, :], [P, VW])
                    for k in range(KCH)]
            wp_t = [load(f"wp{c}", wp_d[c * P:(c + 1) * P, :], [P, C])
                    for c in range(2)]
            load_x(2)
            load_x(3)

            epsq = sb.tile([2, 1], f32, tag="epsq", name="epsq")
            nc.gpsimd.memset(epsq[:], RMS_EPS)
            epsk = sb.tile([P, 1], f32, tag="epsk", name="epsk")
            nc.gpsimd.memset(epsk[:], 64.0 * RMS_EPS)

            # persistent intermediates
            rcq_t = [sb.tile([P, T], f16, tag=f"rcq{c}", name=f"rcq{c}")
                     for c in range(2)]
            rck_t = [sb.tile([P, T], f16, tag=f"rck{c}", name=f"rck{c}")
                     for c in range(2)]
            yT_t = [sb.tile([P, T], f16, tag=f"yT{c}", name=f"yT{c}")
                    for c in range(2)]
            v_t = [sb.tile([P, VW], f16, tag=f"v{s}", name=f"v{s}")
                   for s in range(T // P)]
            invkT_t = [[sb.tile([P, 8], f32, tag=f"ikT{n}_{c}", name=f"ikT{n}_{c}")
                        for c in range(2)] for n in range(NB)]

            # ---------------- phase-1 pieces ----------------
            def p1_qk(n, which):
                """Projection + stats + rope for q (which='q') or k ('k')."""
                nsl = slice(n * TB, (n + 1) * TB)
                w_t = wq_t if which == "q" else wk_t
                pp = [ps.tile([P, TB], f32, tag="p1", name=f"p{which}{n}_{c}",
                              bufs=2) for c in range(2)]
                for c in range(2):
                    for k in range(KCH):
                        nc.tensor.matmul(pp[c][:],
                                         lhsT=w_t[k][:, c * P:(c + 1) * P],
                                         rhs=x_t[k][n][:], start=(k == 0),
                                         stop=(k == KCH - 1))
                xm = tr.tile([P, 2 * TB], f16, tag="xm", name=f"x{which}{n}",
                             bufs=2)
                for c in range(2):
                    nc.scalar.activation(xm[:, c * TB:(c + 1) * TB], pp[c][:],
                                         Copy)
                sq = tr.tile([P, 2 * TB], f16, tag="sqm", name=f"sq{which}{n}",
                             bufs=2)
                nc.vector.tensor_mul(sq[:], xm[:], xm[:])

                if which == "q":
                    src = tr.tile([P, 2 * TB], f16, tag="xnm", name=f"xn{n}",
                                  bufs=2)
                    for c in range(2):
                        csl = slice(c * TB, (c + 1) * TB)
                        msum = ps.tile([2, TB], f32, tag="pst",
                                       name=f"ms{n}_{c}", bufs=2)
                        nc.tensor.matmul(msum[:], lhsT=ind2_t[:],
                                         rhs=sq[:, csl], start=True, stop=True)
                        invr = tr.tile([2, TB], f16, tag="invr",
                                       name=f"ivr{n}_{c}", bufs=2)
                        nc.scalar.activation(invr[:], msum[:], Ln,
                                             bias=epsq[:], scale=1.0 / 64.0)
                        nc.scalar.activation(invr[:], invr[:], Exp, scale=-0.5)
                        inv128 = ps.tile([P, TB], f32, tag="p1",
                                         name=f"iv{n}_{c}", bufs=2)
                        nc.tensor.matmul(inv128[:], lhsT=bc64_t[:],
                                         rhs=invr[:], start=True, stop=True)
                        nc.vector.tensor_mul(src[:, csl], xm[:, csl],
                                             inv128[:])
                else:
                    src = xm
                    for c in range(2):
                        kst = ps.tile([P, 8], f32, tag="pst",
                                      name=f"kst{n}_{c}", bufs=2)
                        for sc in range(4):
                            nc.tensor.matmul(
                                kst[:, 2 * sc:2 * sc + 2],
                                lhsT=sq[:, c * TB + sc * P:c * TB + (sc + 1) * P],
                                rhs=ind2_t[:], start=True, stop=True)
                        nc.scalar.activation(invkT_t[n][c][:], kst[:], Ln,
                                             bias=epsk[:])
                        nc.scalar.activation(invkT_t[n][c][:],
                                             invkT_t[n][c][:], Exp, scale=-0.5)

                xsw = tr.tile([P, 2 * TB], f16, tag="xsw",
                              name=f"x{which}sw{n}", bufs=2)
                for g in range(2):
                    a, b = 64 * g, 64 * g + 32
                    nc.sync.dma_start(out=xsw[a:a + 32, :], in_=src[b:b + 32, :])
                    nc.sync.dma_start(out=xsw[b:b + 32, :], in_=src[a:a + 32, :])
                rc_t = rcq_t if which == "q" else rck_t
                for c in range(2):
                    csl = slice(c * TB, (c + 1) * TB)
                    t1 = tr.tile([P, TB], f16, tag="t12",
                                 name=f"t1{which}{n}_{c}", bufs=3)
                    t2 = tr.tile([P, TB], f16, tag="t12",
                                 name=f"t2{which}{n}_{c}", bufs=3)
                    nc.vector.tensor_mul(t1[:], src[:, csl], cos2_t[:, nsl])
                    nc.vector.tensor_mul(t2[:], xsw[:, csl], sin2_t[:, nsl])
                    nc.vector.tensor_add(rc_t[c][:, nsl], t1[:], t2[:])

            def p1_v(n):
                for s_rel in range(4):
                    pv = ps.tile([P, VW], f32, tag="p1", name=f"pv{n}_{s_rel}",
                                 bufs=2)
                    for k in range(KCH):
                        nc.tensor.matmul(
                            pv[:], lhsT=x_t[k][n][:, s_rel * P:(s_rel + 1) * P],
                            rhs=wv_t[k][:], start=(k == 0), stop=(k == KCH - 1))
                    vt = v_t[4 * n + s_rel]
                    nc.vector.tensor_copy(vt[:], pv[:])
                    nc.vector.tensor_scalar(vt[:, 64:VW:65], pv[:, 64:VW:65],
                                            0.0, 1.0, Alu.mult, Alu.add)

            # ---------------- attention ----------------
            def attn_head(h, j, den4):
                """S->exp->PV software-pipelined; den row lands in den4[h]."""
                cch, half = h // 2, h % 2
                rsl = slice(64 * half, 64 * half + 64)
                n_k = 4 * (j + 1)
                Y = ps.tile([65, TB], f32, tag="py", name=f"Y{h}_{j}", bufs=2)
                pend = None  # (e0, mt, k)
                for k in range(n_k):
                    r = k - 4 * j
                    mt = 128 * r if r > 0 else 0
                    S = ps.tile([P, TB], f32, tag="ps", name=f"S{h}_{j}_{k}",
                                bufs=2)
                    nc.tensor.matmul(
                        S[:, mt:], lhsT=rck_t[cch][rsl, k * P:(k + 1) * P],
                        rhs=rcq_t[cch][rsl, j * TB + mt:(j + 1) * TB],
                        start=True, stop=True)
                    e0 = tr.tile([P, TB], f16, tag="e0", name=f"e{h}_{j}_{k}",
                                 bufs=3)
                    nc.scalar.activation(
                        e0[:, mt:], S[:, mt:], Exp,
                        scale=invkT_t[k // 4][cch][:, 2 * (k % 4) + half:
                                                   2 * (k % 4) + half + 1])
                    if r >= 0:
                        nc.gpsimd.affine_select(
                            out=e0[:, 128 * r:128 * r + 128],
                            in_=e0[:, 128 * r:128 * r + 128],
                            pattern=[[1, 128]], compare_op=Alu.is_ge,
                            fill=0.0, base=0, channel_multiplier=-1)
                    if pend is not None:
                        pe0, pmt, pk = pend
                        nc.tensor.matmul(Y[:, pmt:],
                                         lhsT=v_t[pk][:, 65 * h:65 * h + 65],
                                         rhs=pe0[:, pmt:], start=(pk == 0),
                                         stop=False)
                    pend = (e0, mt, k)
                pe0, pmt, pk = pend
                nc.tensor.matmul(Y[:, pmt:], lhsT=v_t[pk][:, 65 * h:65 * h + 65],
                                 rhs=pe0[:, pmt:], start=(pk == 0), stop=True)
                # y rows to sbuf (f32: pre-normalization values can be large),
                # denominator row into the window-shared den4 tile.
                yraw = tr.tile([65, TB], f32, tag="yrw", name=f"yr{h}_{j}",
                               bufs=5, padded_shape=[P, TB])
                nc.vector.tensor_copy(yraw[:], Y[:])
                nc.sync.dma_start(out=den4[h:h + 1, :], in_=yraw[64:65, :])
                return yraw

            def attn_tail(j, den4, yraws):
                """Batched denominator reciprocal + normalize + yT scatter."""
                jsl = slice(j * TB, (j + 1) * TB)
                invd = tr.tile([HPC, TB], f16, tag="invd", name=f"invd{j}",
                               bufs=2)
                nc.scalar.activation(invd[:], den4[:], Ln)
                nc.scalar.activation(invd[:], invd[:], Exp, scale=-1.0)
                for h in range(HPC):
                    cch, half = h // 2, h % 2
                    rsl = slice(64 * half, 64 * half + 64)
                    bcD = ps.tile([64, TB], f32, tag="ps", name=f"bcD{h}_{j}",
                                  bufs=2)
                    nc.tensor.matmul(bcD[:], lhsT=sel4_t[:, 64 * h:64 * h + 64],
                                     rhs=invd[:], start=True, stop=True)
                    yn = tr.tile([64, TB], f16, tag="ynm", name=f"yn{h}_{j}",
                                 bufs=2, padded_shape=[P, TB])
                    nc.vector.tensor_mul(yn[:], yraws[h][0:64, :], bcD[:])
                    nc.sync.dma_start(out=yT_t[cch][rsl, jsl], in_=yn[:])

            # ---------------- out-projection ----------------
            def p4(j):
                jsl = slice(j * TB, (j + 1) * TB)
                for o in range(8):
                    osl = slice(o * P, (o + 1) * P)
                    po = ps.tile([P, TB], f32, tag="p1", name=f"po{j}_{o}",
                                 bufs=2)
                    nc.tensor.matmul(po[:], lhsT=wp_t[0][:, osl],
                                     rhs=yT_t[0][:, jsl], start=True, stop=False)
                    nc.tensor.matmul(po[:], lhsT=wp_t[1][:, osl],
                                     rhs=yT_t[1][:, jsl], start=False, stop=True)
                    ob = tr.tile([P, TB], f16, tag="ob", name=f"ob{j}_{o}",
                                 bufs=3)
                    nc.vector.tensor_copy(ob[:], po[:])
                    nc.sync.dma_start(out=out_d[osl, jsl], in_=ob[:])

            # ---------------- schedule ----------------
            # window 0: projections for block 0 only
            p1_qk(0, "q")
            p1_qk(0, "k")
            p1_v(0)
            # windows 1..NB: attention j = w-1 interleaved with p1(w) pieces
            for w in range(1, NB + 1):
                j = w - 1
                den4 = tr.tile([HPC, TB], f32, tag="den4", name=f"den4_{j}",
                               bufs=2)
                yraws = []
                yraws.append(attn_head(0, j, den4))
                if w < NB:
                    p1_qk(w, "q")
                yraws.append(attn_head(1, j, den4))
                if w < NB:
                    p1_qk(w, "k")
                yraws.append(attn_head(2, j, den4))
                if w < NB:
                    p1_v(w)
                yraws.append(attn_head(3, j, den4))
                attn_tail(j, den4, yraws)
                if w >= 2:
                    p4(w - 2)        # out-projection one window behind
            p4(NB - 1)

    nc.compile()
    return nc


def _get_module():
    if "nc" not in _CACHE:
        _CACHE["nc"] = _build_module()
        _CACHE["consts"] = _build_consts()
    return _CACHE["nc"], _CACHE["consts"]


def _core_inputs(x, w_q, w_k, w_v, w_proj, core):
    """Build the per-core input map (numpy fp16, host-side sharding)."""
    b = core // 4
    g = core % 4
    heads = [4 * g + j for j in range(HPC)]

    xt = np.ascontiguousarray(x[b].T).astype(np.float16)     # [C, T]

    # attention-layout column perm: col m of chunk c -> head 2c+(m//64), dim m%64
    perm = np.empty(256, dtype=np.int64)
    for m in range(256):
        c, mm = m // 128, m % 128
        perm[m] = 64 * heads[2 * c + mm // 64] + (mm % 64)
    wq = np.ascontiguousarray(w_q[perm, :].T).astype(np.float16)   # [C, 256]
    wk = np.ascontiguousarray(w_k[perm, :].T).astype(np.float16)

    wv_aug = np.zeros((C, VW), dtype=np.float32)
    for j in range(HPC):
        wv_aug[:, 65 * j:65 * j + 64] = w_v[64 * heads[j]:64 * heads[j] + 64, :].T
    wv = wv_aug.astype(np.float16)

    wp = np.ascontiguousarray(w_proj[:, perm].T).astype(np.float16)  # [256, C]

    return dict(xt=xt, wq=wq, wk=wk, wv=wv, wp=wp)


def kernel(x, w_q, w_k, w_v, w_proj, _trace=False, _trace_cores=None):
    from concourse.bass_utils import run_bass_kernel_spmd

    nc, consts = _get_module()
    x = np.asarray(x, dtype=np.float32)
    in_maps = []
    for core in range(N_CORES):
        m = _core_inputs(np.asarray(x), np.asarray(w_q), np.asarray(w_k),
                         np.asarray(w_v), np.asarray(w_proj), core)
        m.update(consts)
        in_maps.append(m)

    res = run_bass_kernel_spmd(nc, in_maps, list(range(N_CORES)),
                               trace=_trace, trace_cores=_trace_cores)
    outs = [res.results[c]["outT"] for c in range(N_CORES)]
    out = np.empty((B, T, C), dtype=np.float32)
    for b in range(B):
        acc = outs[4 * b].astype(np.float32)
        for g in range(1, 4):
            acc = acc + outs[4 * b + g].astype(np.float32)
        out[b] = acc.T
    if _trace:
        kernel._last_exec_time_ns = res.exec_time_ns
        kernel._last_results = res
    return out
